# revision 13
# baseline (speedup 1.0000x reference)
"""VQ codebook (nn_ActionVQ): idx = argmin_a ||e - W_a||^2, q = W[idx].

Strategy (data-parallel over 8 NeuronCores; W replicated):
  - shard e along N (4096 rows/core). Per 128-row m-tile on each core:
      PE:  raw scores s = e @ W^T in fp16 (64 matmuls: K=128 x4, N=512 x16)
           accumulated fp32 in PSUM (4 double-buffered 2048-wide quarters).
      ACT: evicts PSUM -> SBUF (plain copy; ScalarE is otherwise idle).
      DVE: InstMax (top-8 raw values) + InstMaxIndex (their indices) over
           the full 8192-wide row.
      GPSIMD: indirect-DMA gathers q rows from W in HBM by raw-argmax index.
  - host: corrected scores c = v8 - 0.5*||w||^2[i8] over the 8 candidates
    (the row-constant ||e||^2 does not move the argmin), picks the best,
    and exactly recomputes rows whose top-2 corrected margin is below the
    fp16 noise bound DELTA (~1% of rows) plus rows where the corrected
    argmax differs from the raw argmax used for the device-side gather.
"""

import os
import numpy as np

N_TOTAL, A, D = 32768, 8192, 512
NCORES = 8
NLOC = N_TOTAL // NCORES          # 4096 rows per core
MT = NLOC // 128                  # 32 m-tiles per core
# |score| error bound for fp16 inputs: per-element sigma ~2e-4, observed
# max ~1e-3. Corrected-margin below DELTA => exact host recompute.
DELTA = 2.5e-3

_CACHE = {}


def _build_nc():
    import concourse.bacc as bacc
    import concourse.tile as tile
    import concourse.mybir as mybir
    from concourse import bass

    dt = mybir.dt
    nc = bacc.Bacc("TRN2", target_bir_lowering=False, debug=False,
                   num_devices=NCORES)

    eT_d = nc.dram_tensor("eT", [4, 128, NLOC], dt.float16, kind="ExternalInput").ap()
    WT_d = nc.dram_tensor("WT", [4, 128, A], dt.float16, kind="ExternalInput").ap()
    Wf_d = nc.dram_tensor("Wf", [A, D], dt.float32, kind="ExternalInput").ap()
    v8_o = nc.dram_tensor("v8", [MT, 128, 8], dt.float32, kind="ExternalOutput").ap()
    i8_o = nc.dram_tensor("i8", [MT, 128, 8], dt.uint32, kind="ExternalOutput").ap()
    q_o = nc.dram_tensor("q", [NLOC, D], dt.float32, kind="ExternalOutput").ap()

    with tile.TileContext(nc) as tc:
        with tc.tile_pool(name="res", bufs=1) as res, \
             tc.tile_pool(name="etp", bufs=2) as etp, \
             tc.tile_pool(name="scp", bufs=2) as scp, \
             tc.tile_pool(name="smp", bufs=2) as smp, \
             tc.tile_pool(name="qtp", bufs=2) as qtp, \
             tc.tile_pool(name="ps", bufs=2, space="PSUM") as pp:
            wt = res.tile([128, 4, A], dt.float16, tag="wt")
            # Column-major codebook loads (still 4 DMAs): DMA queues drain
            # FIFO, so the first load (cols 0:2048, all k) completes in
            # ~1/4 the time and tile 0's first PSUM quarter starts early.
            for qcol in range(4):
                c0 = qcol * 2048
                src = WT_d[:, :, c0:c0 + 2048].rearrange("k p c -> p k c")
                nc.sync.dma_start(wt[:, :, c0:c0 + 2048], src)

            for m in range(MT):
                et = etp.tile([128, 4, 128], dt.float16, tag="et")
                for k in range(4):
                    nc.sync.dma_start(et[:, k, :], eT_d[k, :, m * 128:(m + 1) * 128])
                sc = scp.tile([128, A], dt.float32, tag="sc")
                for quarter in range(4):
                    pq = pp.tile([128, 2048], dt.float32, tag="pq",
                                 name=f"pq_{m}_{quarter}")
                    for k in range(4):
                        for c in range(4):
                            n0 = c * 512
                            nc.tensor.matmul(pq[:, n0:n0 + 512], lhsT=et[:, k, :],
                                             rhs=wt[:, k, quarter * 2048 + n0:
                                                    quarter * 2048 + n0 + 512],
                                             start=(k == 0), stop=(k == 3))
                    q0 = quarter * 2048
                    nc.scalar.copy(sc[:, q0:q0 + 2048], pq[:])
                v8t = smp.tile([128, 8], dt.float32, tag="v8t")
                i8t = smp.tile([128, 8], dt.uint32, tag="i8t")
                nc.vector.max(v8t[:], sc[:])
                nc.vector.max_index(i8t[:], v8t[:], sc[:])
                qt = qtp.tile([128, D], dt.float32, tag="qt")
                nc.gpsimd.indirect_dma_start(
                    out=qt[:], out_offset=None, in_=Wf_d[:],
                    in_offset=bass.IndirectOffsetOnAxis(ap=i8t[:, 0:1], axis=0))
                nc.sync.dma_start(v8_o[m, :, :], v8t[:])
                nc.sync.dma_start(i8_o[m, :, :], i8t[:])
                nc.sync.dma_start(q_o[m * 128:(m + 1) * 128, :], qt[:])
    nc.compile()
    return nc


def kernel(e, W):
    from concourse.bass_utils import run_bass_kernel_spmd

    e = np.ascontiguousarray(np.asarray(e), dtype=np.float32)
    W = np.ascontiguousarray(np.asarray(W), dtype=np.float32)
    assert e.shape == (N_TOTAL, D) and W.shape == (A, D)

    if "nc" not in _CACHE:
        _CACHE["nc"] = _build_nc()
    nc = _CACHE["nc"]

    WT16 = np.ascontiguousarray(W.T.astype(np.float16)).reshape(4, 128, A)
    in_maps = []
    for c in range(NCORES):
        ec = e[c * NLOC:(c + 1) * NLOC]
        eT16 = np.ascontiguousarray(ec.T.astype(np.float16)).reshape(4, 128, NLOC)
        in_maps.append({"eT": eT16, "WT": WT16, "Wf": W})

    trace = os.environ.get("VQ_KERNEL_TRACE", "") == "1"
    res = run_bass_kernel_spmd(nc, in_maps, core_ids=list(range(NCORES)),
                               trace=trace)
    if trace:
        _CACHE["last_result"] = res

    v8 = np.concatenate([r["v8"].reshape(NLOC, 8) for r in res.results], 0)
    i8 = np.concatenate([r["i8"].reshape(NLOC, 8) for r in res.results], 0)
    q = np.concatenate([r["q"] for r in res.results], 0)

    # Host-side codeword-norm correction over the top-8 raw candidates.
    # Defensive: treat any out-of-range index slot (e.g. a hardware
    # "unmatched" sentinel) as an invalid candidate.
    invalid = i8 >= A
    i8 = np.where(invalid, 0, i8)
    w2h = 0.5 * (W * W).sum(1)                   # fp32 [A]
    c8 = v8 - w2h[i8]                            # corrected candidate scores
    c8[invalid] = -np.inf
    j = np.argmax(c8, axis=1)
    rows = np.arange(N_TOTAL)
    idx = i8[rows, j].astype(np.int32)
    top1 = c8[rows, j]
    c8[rows, j] = -np.inf
    margin = top1 - c8.max(axis=1)

    # Device q used the raw argmax (candidate 0); fix rows where the
    # corrected argmax differs.
    moved = np.where(j != 0)[0]
    if moved.size:
        q[moved] = W[idx[moved]]

    # Exact fp32 recompute where the fp16 margin is ambiguous (or any
    # candidate slot was invalid).
    flagged = np.where((margin < DELTA) | invalid.any(axis=1))[0]
    if flagged.size:
        ef = e[flagged]
        d2 = ((ef * ef).sum(1, keepdims=True)
              - 2.0 * (ef @ W.T)
              + (W * W).sum(1)[None, :])
        idx_f = np.argmin(d2, axis=1).astype(np.int32)
        idx[flagged] = idx_f
        q[flagged] = W[idx_f]
    return idx, q


# revision 15
# speedup vs baseline: 1.0057x; 1.0057x over previous
"""VQ codebook (nn_ActionVQ): idx = argmin_a ||e - W_a||^2, q = W[idx].

Strategy (data-parallel over 8 NeuronCores; W replicated):
  - shard e along N (4096 rows/core). Per 128-row m-tile on each core:
      PE:  raw scores s = e @ W^T in fp16 (64 matmuls: K=128 x4, N=512 x16)
           accumulated fp32 in PSUM (4 double-buffered 2048-wide quarters).
      ACT: evicts PSUM -> SBUF (plain copy; ScalarE is otherwise idle).
      DVE: InstMax (top-8 raw values) + InstMaxIndex (their indices) over
           the full 8192-wide row.
      GPSIMD: indirect-DMA gathers q rows from W in HBM by raw-argmax index.
  - host: corrected scores c = v8 - 0.5*||w||^2[i8] over the 8 candidates
    (the row-constant ||e||^2 does not move the argmin), picks the best,
    and exactly recomputes rows whose top-2 corrected margin is below the
    fp16 noise bound DELTA (~1% of rows) plus rows where the corrected
    argmax differs from the raw argmax used for the device-side gather.
"""

import os
import numpy as np

N_TOTAL, A, D = 32768, 8192, 512
NCORES = 8
NLOC = N_TOTAL // NCORES          # 4096 rows per core
MT = NLOC // 128                  # 32 m-tiles per core
# |score| error bound for fp16 inputs: per-element sigma ~2e-4, observed
# max ~1e-3. Corrected-margin below DELTA => exact host recompute.
DELTA = 2.5e-3

_CACHE = {}


def _build_nc():
    import concourse.bacc as bacc
    import concourse.tile as tile
    import concourse.mybir as mybir
    from concourse import bass

    dt = mybir.dt
    nc = bacc.Bacc("TRN2", target_bir_lowering=False, debug=False,
                   num_devices=NCORES)

    eT_d = nc.dram_tensor("eT", [4, 128, NLOC], dt.float16, kind="ExternalInput").ap()
    WT_d = nc.dram_tensor("WT", [4, 128, A], dt.float16, kind="ExternalInput").ap()
    Wf_d = nc.dram_tensor("Wf", [A, D], dt.float32, kind="ExternalInput").ap()
    v8_o = nc.dram_tensor("v8", [MT, 128, 8], dt.float32, kind="ExternalOutput").ap()
    i8_o = nc.dram_tensor("i8", [MT, 128, 8], dt.uint32, kind="ExternalOutput").ap()
    q_o = nc.dram_tensor("q", [NLOC, D], dt.float32, kind="ExternalOutput").ap()

    with tile.TileContext(nc) as tc:
        with tc.tile_pool(name="res", bufs=1) as res, \
             tc.tile_pool(name="etp", bufs=2) as etp, \
             tc.tile_pool(name="scp", bufs=2) as scp, \
             tc.tile_pool(name="smp", bufs=2) as smp, \
             tc.tile_pool(name="qtp", bufs=2) as qtp, \
             tc.tile_pool(name="ps", bufs=2, space="PSUM") as pp:
            wt = res.tile([128, 4, A], dt.float16, tag="wt")
            # Column-major codebook loads (still 4 DMAs): DMA queues drain
            # FIFO, so the first load (cols 0:2048, all k) completes in
            # ~1/4 the time and tile 0's first PSUM quarter starts early.
            for qcol in range(4):
                c0 = qcol * 2048
                src = WT_d[:, :, c0:c0 + 2048].rearrange("k p c -> p k c")
                nc.sync.dma_start(wt[:, :, c0:c0 + 2048], src)

            for m in range(MT):
                et = etp.tile([128, 4, 128], dt.float16, tag="et")
                for k in range(4):
                    nc.sync.dma_start(et[:, k, :], eT_d[k, :, m * 128:(m + 1) * 128])
                sc = scp.tile([128, A], dt.float32, tag="sc")
                for quarter in range(4):
                    pq = pp.tile([128, 2048], dt.float32, tag="pq",
                                 name=f"pq_{m}_{quarter}")
                    for k in range(4):
                        for c in range(4):
                            n0 = c * 512
                            nc.tensor.matmul(pq[:, n0:n0 + 512], lhsT=et[:, k, :],
                                             rhs=wt[:, k, quarter * 2048 + n0:
                                                    quarter * 2048 + n0 + 512],
                                             start=(k == 0), stop=(k == 3))
                    q0 = quarter * 2048
                    nc.scalar.copy(sc[:, q0:q0 + 2048], pq[:])
                v8t = smp.tile([128, 8], dt.float32, tag="v8t")
                i8t = smp.tile([128, 8], dt.uint32, tag="i8t")
                nc.vector.max(v8t[:], sc[:])
                nc.vector.max_index(i8t[:], v8t[:], sc[:])
                nc.sync.dma_start(v8_o[m, :, :], v8t[:])
                nc.sync.dma_start(i8_o[m, :, :], i8t[:])
                if m < MT - 1:
                    # Gather q = W[argmax] on-device; the LAST tile's gather
                    # chain would sit on the critical tail, so its 128 rows
                    # are filled host-side instead (host rewrites q rows
                    # anyway for corrected/flagged indices).
                    qt = qtp.tile([128, D], dt.float32, tag="qt")
                    nc.gpsimd.indirect_dma_start(
                        out=qt[:], out_offset=None, in_=Wf_d[:],
                        in_offset=bass.IndirectOffsetOnAxis(ap=i8t[:, 0:1], axis=0))
                    nc.sync.dma_start(q_o[m * 128:(m + 1) * 128, :], qt[:])
    nc.compile()
    return nc


def kernel(e, W):
    from concourse.bass_utils import run_bass_kernel_spmd

    e = np.ascontiguousarray(np.asarray(e), dtype=np.float32)
    W = np.ascontiguousarray(np.asarray(W), dtype=np.float32)
    assert e.shape == (N_TOTAL, D) and W.shape == (A, D)

    if "nc" not in _CACHE:
        _CACHE["nc"] = _build_nc()
    nc = _CACHE["nc"]

    WT16 = np.ascontiguousarray(W.T.astype(np.float16)).reshape(4, 128, A)
    in_maps = []
    for c in range(NCORES):
        ec = e[c * NLOC:(c + 1) * NLOC]
        eT16 = np.ascontiguousarray(ec.T.astype(np.float16)).reshape(4, 128, NLOC)
        in_maps.append({"eT": eT16, "WT": WT16, "Wf": W})

    trace = os.environ.get("VQ_KERNEL_TRACE", "") == "1"
    res = run_bass_kernel_spmd(nc, in_maps, core_ids=list(range(NCORES)),
                               trace=trace)
    if trace:
        _CACHE["last_result"] = res

    v8 = np.concatenate([r["v8"].reshape(NLOC, 8) for r in res.results], 0)
    i8 = np.concatenate([r["i8"].reshape(NLOC, 8) for r in res.results], 0)
    q = np.concatenate([r["q"] for r in res.results], 0)

    # Host-side codeword-norm correction over the top-8 raw candidates.
    # Defensive: treat any out-of-range index slot (e.g. a hardware
    # "unmatched" sentinel) as an invalid candidate.
    invalid = i8 >= A
    i8 = np.where(invalid, 0, i8)
    w2h = 0.5 * (W * W).sum(1)                   # fp32 [A]
    c8 = v8 - w2h[i8]                            # corrected candidate scores
    c8[invalid] = -np.inf
    j = np.argmax(c8, axis=1)
    rows = np.arange(N_TOTAL)
    idx = i8[rows, j].astype(np.int32)
    top1 = c8[rows, j]
    c8[rows, j] = -np.inf
    margin = top1 - c8.max(axis=1)

    # Each core's last m-tile skips the device-side gather (tail latency);
    # fill those 128 rows per core from the final indices.
    last_rows = (np.arange(NCORES)[:, None] * NLOC
                 + (MT - 1) * 128 + np.arange(128)[None, :]).ravel()
    q[last_rows] = W[idx[last_rows]]

    # Device q used the raw argmax (candidate 0); fix rows where the
    # corrected argmax differs.
    moved = np.where(j != 0)[0]
    if moved.size:
        q[moved] = W[idx[moved]]

    # Exact fp32 recompute where the fp16 margin is ambiguous (or any
    # candidate slot was invalid).
    flagged = np.where((margin < DELTA) | invalid.any(axis=1))[0]
    if flagged.size:
        ef = e[flagged]
        d2 = ((ef * ef).sum(1, keepdims=True)
              - 2.0 * (ef @ W.T)
              + (W * W).sum(1)[None, :])
        idx_f = np.argmin(d2, axis=1).astype(np.int32)
        idx[flagged] = idx_f
        q[flagged] = W[idx_f]
    return idx, q
